# revision 33
# baseline (speedup 1.0000x reference)
"""HarmonyGenerator Trainium2 kernel.

Math: the reference's 3x3 conv on [T,1,1,D] degenerates to a 3-tap conv along
the feature axis (only the kernel's middle row touches data).  Conv and the
three linear heads are both linear, so the conv folds into the head weights
(W' = 3-tap correlation of W along K) and the constant context-embedding rows
plus conv bias fold into the output bias.  The device work is one GEMM:

    out[2048, 168] = [melody | lyrics][2048, 50681] @ W'[50681, 168] + bias

Sharding: K (feature) axis split 8 ways, 6336 rows per core (49 full
128-row k-tiles + one 64-row tail tile; 8*6336 = 50688 >= 50681, 7 pad rows
on the last core only).  Each core reads 1/8 of x AND 1/8 of W and produces
a partial [168, 2048] (fp16); partials are summed on the host during the
gather/unshard step.

Schedule (v5): the PE's work rate (1296 ns/k-tile: four full-width 512-col
matmul streams for the first 128 out-cols + two concurrent 40-col pairs) is
~7% faster than the 16-queue DMA delivery rate (~1387 ns/k-tile at the
~409 B/ns aggregate wire speed), so the kernel should finish one chunk after
the last x byte.  What breaks that is delivery ORDER: each HWDGE ring is
FIFO with ~4 DIRECT2D issue credits, and the SDMA engines split bandwidth
~50/50 between the rings, so when two consecutively-needed chunks sit on
the same ring the PE waits ~5-12 us for the second one while the other ring
streams bytes it won't need for a while; the stall also re-trips the HAM
clock gate (>3.4 us PE-idle drops the array to 1.2 GHz for the next ~3.4 us
of work).  v5 therefore issues x as uniform 2-k-tile (1 MB) chunks strictly
alternating between the rings, and splits every W chunk into two half-DMAs
(one per ring), so both rings drain in lock-step with PE need order and
every chunk lands at the full aggregate rate.  A 24-matmul warm-up burst
(~7 us) delays the real stream until a few chunks of backlog are resident
and keeps the array active from the first cycle, so HAM never re-gates.
The PE consumes chunks in 2-chunk groups (the 40-col pairs of the previous
group run first and fill any arrival wait; the PE is in-order, so filler
placed after a stalling matmul could never slide forward).  The tail keeps
the last chunks at 1 k-tile and interleaves the kt-49 matmuls with PSUM
evictions so only ~2 us of work trails the last byte.

Fixed costs in the measured window: ~1.3 us of framework preamble before
the tile body can issue its first DMA, and ~13 us after the last real
matmul (eviction/store tail, exit barrier, then the NEFF's fixed
254-semaphore clear chain, whose slowest engine -- the Tensor NX at
~131 ns/clear -- defines the measured exec end).
"""

import os
import numpy as np

import concourse.bacc as bacc
import concourse.mybir as mybir
from concourse.tile import TileContext
from concourse.bass_utils import run_bass_kernel_spmd

# Problem shapes (hardcoded per contract)
T = 2048               # steps = length * 128
D_IN = 50937           # 256 ctx + 256 melody/vel + 50425 lyrics
K_GEMM = 50681         # melody(256) + lyrics(50425) features in the GEMM
N_OUT = 168            # 24 chord + 16 beat + 128 mel
N_CORES = 8
K_PER = 6336           # per-core K rows (49*128 + 64; 8*6336 = 50688 >= 50681)
KT = 50                # k-tiles per core (49 full 128-row + one 64-row tail)
TB = 512               # t-block (max fp32 moving dim / PSUM bank)
NTB = T // TB          # 4
KT_LAST = KT - 1

WARMUP_MMS = 12
# Dummy matmuls interleaved at the start of early PE groups (group index ->
# count): they have no chunk dependency, so when the next group's mels would
# stall on a late chunk arrival they absorb the wait (and keep the HAM clock
# gate warm through it); when the chunk is on time they cost their stream
# time like front warm-up.  Total front + fill matches the tuned 24.
RAMP_FILL = {1: 2, 2: 2, 3: 2, 4: 3, 5: 3}

# x chunks (start_kt, n_kt): uniform 1 MB chunks, strictly alternating
# between the two HWDGE rings so delivery order tracks PE need order
# (smaller chunks starve the ~4-credit-deep ring issue pipeline and cost
# ~5% of delivery rate; larger ones make the arrival frontier lumpy);
# 1-k-tile tail so little work trails the last byte.
X_CHUNKS = [(2 * i, 2) for i in range(24)] + [(48, 1), (49, 1)]
# W chunks are split into two half-DMAs, one per ring, so they don't skew
# the rings' byte balance.  Each lands a couple of x chunks ahead of need.
W_CHUNKS = [(0, 4), (4, 6), (10, 8), (18, 8), (26, 8), (34, 8), (42, 8)]
# issue order: W chunk j is needed from x chunk [0, 2, 5, 9, 13, 17, 21].
ORDER = (
    [('w', 0), ('x', 0), ('x', 1), ('w', 1), ('x', 2), ('x', 3), ('w', 2),
     ('x', 4), ('x', 5), ('x', 6), ('x', 7), ('w', 3), ('x', 8), ('x', 9),
     ('x', 10), ('x', 11), ('w', 4), ('x', 12), ('x', 13), ('x', 14),
     ('x', 15), ('w', 5), ('x', 16), ('x', 17), ('x', 18), ('x', 19),
     ('w', 6), ('x', 20), ('x', 21), ('x', 22), ('x', 23), ('x', 24),
     ('x', 25)]
)
# PE consumption groups (indices into X_CHUNKS): fine in the ramp, paired in
# the body to amortize the ~166 ns column-group reconfig per cb<->mel swap
# (wider groups make the chunk-arrival frontier lumpier and stall the PE at
# group boundaries).
PE_GROUPS = ([[0], [1], [2], [3]] + [[i, i + 1] for i in range(4, 24, 2)]
             + [[24], [25]])

assert sum(n for _, n in X_CHUNKS) == KT
assert sum(n for _, n in W_CHUNKS) == KT
assert [i for g in PE_GROUPS for i in g] == list(range(len(X_CHUNKS)))

_NC = None
LAST_RESULT = None     # BassKernelResults of the most recent run (for test.py)


def _build_nc():
    f32 = mybir.dt.float32
    f16 = mybir.dt.float16
    nc = bacc.Bacc()
    # xt is plain [K, T]: each chunk is then one contiguous 1 MB DRAM extent
    # (a k-tile-major partition-planar pack was tried and is ~2% slower --
    # it turns a chunk into 128 strided 8 KB extents, which HBM likes less).
    xt = nc.dram_tensor("xt", [K_PER, T], f16, kind="ExternalInput")
    w = nc.dram_tensor("w", [128, KT * N_OUT], f16, kind="ExternalInput")
    out = nc.dram_tensor("out", [N_OUT, T], f16, kind="ExternalOutput")

    with TileContext(nc) as tc:
        with (
            tc.tile_pool(name="wp", bufs=1) as wp,
            tc.tile_pool(name="xp", bufs=1) as xp,
            tc.tile_pool(name="op", bufs=2) as op,
            tc.tile_pool(name="ps", bufs=1, space="PSUM") as ps,
        ):
            # HAM warm-up: the PE clock-gate holds matmuls at 1.2 GHz until
            # ~3.4us of sustained activity, and the first chunks need a few
            # us to land.  Burn that window on dummy matmuls so the real
            # stream starts warm with delivered backlog and never pauses
            # long enough to re-gate.
            dm = wp.tile([128, TB], f16, tag="warm", name="warmup")
            nc.vector.memset(dm[:], 0.0)
            ps_warm = ps.tile([128, TB], f32, tag="warm_ps", name="ps_warm")
            for _ in range(WARMUP_MMS):
                nc.tensor.matmul(ps_warm[:], dm[:, 0:128], dm[:], start=True, stop=True)

            # Build W tiles and x tiles; issue DMAs on the two HWDGE rings.
            # x chunk i goes to ring i%2; W chunks go half to each ring.
            w_of = {}
            x_of = {}
            rings = [nc.sync, nc.scalar]
            for kind, idx in ORDER:
                if kind == 'w':
                    s, n = W_CHUNKS[idx]
                    tile = wp.tile([128, n * N_OUT], f16, tag=f"w{idx}", name=f"w{idx}")
                    h = n // 2
                    rings[0].dma_start(
                        tile[:, 0:h * N_OUT],
                        w[:, s * N_OUT:(s + h) * N_OUT])
                    rings[1].dma_start(
                        tile[:, h * N_OUT:n * N_OUT],
                        w[:, (s + h) * N_OUT:(s + n) * N_OUT])
                    for j in range(n):
                        w_of[s + j] = (tile, j * N_OUT)
                else:
                    s, n = X_CHUNKS[idx]
                    r = idx % 2
                    if s == KT_LAST:
                        # 64-row tail k-tile (K rows 6272..6335)
                        tile = xp.tile([64, T], f16, tag="xtail", name="xtail",
                                       bufs=1)
                        rings[r].dma_start(tile[:], xt[s * 128:s * 128 + 64, :])
                    elif n == 1:
                        tile = xp.tile([128, T], f16, tag="x1", name=f"x1_{idx}",
                                       bufs=2)
                        rings[r].dma_start(tile[:], xt[s * 128:(s + 1) * 128, :])
                    else:
                        tile = xp.tile([128, n * T], f16, tag=f"x{n}",
                                       name=f"x{n}_{idx}", bufs=14)
                        rings[r].dma_start(
                            tile[:].rearrange("p (a t) -> p a t", a=n),
                            xt[s * 128:(s + n) * 128, :].rearrange(
                                "(a p) t -> p a t", p=128),
                        )
                    for j in range(n):
                        x_of[s + j] = (tile, j * T)

            # Persistent accumulators: 4 mel banks + 2 shared cb banks.  Each
            # cb bank holds two t-blocks' [40, TB] outputs col-tiled into
            # partitions 0:40 and 64:104 (concurrent matmuls via tile_position).
            psm = [ps.tile([128, TB], f32, tag=f"m{t}", name=f"psm{t}") for t in range(NTB)]
            psc = [ps.tile([128, TB], f32, tag=f"c{p}", name=f"psc{p}") for p in range(NTB // 2)]

            def krows(kt):
                return 64 if kt == KT_LAST else 128

            def rhs_of(kt, t):
                tile, off = x_of[kt]
                return tile[0:krows(kt), off + t * TB: off + (t + 1) * TB]

            def mel(kt, t):
                wt, j = w_of[kt]
                nc.tensor.matmul(psm[t][:], wt[0:krows(kt), j: j + 128],
                                 rhs_of(kt, t),
                                 start=(kt == 0), stop=(kt == KT_LAST))

            def cb_pair(kt, p):
                wt, j = w_of[kt]
                lhs_c = wt[0:krows(kt), j + 128: j + N_OUT]
                nc.tensor.matmul(psc[p][0:40, :], lhs_c, rhs_of(kt, 2 * p),
                                 start=(kt == 0), stop=(kt == KT_LAST),
                                 tile_position=(0, 0))
                nc.tensor.matmul(psc[p][64:104, :], lhs_c, rhs_of(kt, 2 * p + 1),
                                 start=(kt == 0), stop=(kt == KT_LAST),
                                 tile_position=(0, 64))

            # psm evictions stage t-block pairs into one [128, 2*TB] tile so
            # the out DMA issues once per pair: each DMA_DIRECT2D costs
            # ~0.7 us of descriptor generation on its ring engine, and the
            # tail is issue-bound, not transfer-bound.
            om = [op.tile([128, 2 * TB], f16, tag=f"om{h}", name=f"om{h}", bufs=1)
                  for h in range(2)]

            def evict_mel(t):
                h, half = t // 2, t % 2
                o = om[h][:, half * TB:(half + 1) * TB]
                if half == 0:
                    nc.vector.tensor_copy(o, psm[t][:])
                else:
                    nc.scalar.copy(o, psm[t][:])
                    ring = nc.sync if h == 0 else nc.scalar
                    ring.dma_start(out[0:128, h * 2 * TB:(h + 1) * 2 * TB], om[h][:])

            def evict_cb(p):
                # cb out DMAs go on the SWDGE ring: input traffic is done by
                # now, and each DMA_DIRECT2D issue costs ~0.7 us on its
                # engine -- spreading the eviction issues over three engines
                # (sync/scalar for mel, gpsimd for cb) shortens the tail.
                o = op.tile([104, TB], f16, tag="oc", name=f"oc{p}")
                if p == 0:
                    nc.vector.tensor_copy(o[0:104, :], psc[p][0:104, :])
                else:
                    nc.scalar.copy(o[0:104, :], psc[p][0:104, :])
                nc.gpsimd.dma_start(out[128:N_OUT, 2 * p * TB:(2 * p + 1) * TB], o[0:40, :])
                ring = nc.sync if p == 0 else nc.scalar
                ring.dma_start(out[128:N_OUT, (2 * p + 1) * TB:(2 * p + 2) * TB], o[64:104, :])

            # The cb pairs of group g run right before the mels of group
            # g+1: pairs touch only already-resident data, so when the mels
            # would stall on a fresh chunk arrival the PE fills the wait
            # with pair work instead of idling (the PE is in-order, so
            # pairs placed after stalling mels could never slide forward).
            def kts_of(group):
                return [kt for ci in group
                        for kt in range(X_CHUNKS[ci][0],
                                        X_CHUNKS[ci][0] + X_CHUNKS[ci][1])]

            for gi, group in enumerate(PE_GROUPS):
                if gi > 0:
                    for kt in kts_of(PE_GROUPS[gi - 1]):
                        cb_pair(kt, 0)
                        cb_pair(kt, 1)
                for _ in range(RAMP_FILL.get(gi, 0)):
                    nc.tensor.matmul(ps_warm[:], dm[:, 0:128], dm[:],
                                     start=True, stop=True)
                if gi < len(PE_GROUPS) - 1:
                    for kt in kts_of(group):
                        for t in range(NTB):
                            mel(kt, t)
                else:
                    # kt49: cb pairs first so their evictions overlap the
                    # mel evictions; then mel bank-by-bank with evictions
                    # interleaved so little work trails the last matmul.
                    cb_pair(KT_LAST, 0)
                    cb_pair(KT_LAST, 1)
                    mel(KT_LAST, 0)
                    evict_mel(0)
                    mel(KT_LAST, 1)
                    evict_mel(1)
                    evict_cb(0)
                    mel(KT_LAST, 2)
                    evict_mel(2)
                    mel(KT_LAST, 3)
                    evict_mel(3)
                    evict_cb(1)
    return nc


def _get_nc():
    global _NC
    if _NC is None:
        _NC = _build_nc()
        if not _NC.is_finalized():
            _NC.finalize()
    return _NC


def kernel(**inputs):
    global LAST_RESULT
    melody = np.ascontiguousarray(np.asarray(inputs["melody_tensor"], dtype=np.float32))
    lyrics = np.ascontiguousarray(np.asarray(inputs["lyrics_tensor"], dtype=np.float32))
    emb = np.asarray(inputs["emb"], dtype=np.float32)
    conv_w = np.asarray(inputs["conv_w"], dtype=np.float32)
    conv_b = np.asarray(inputs["conv_b"], dtype=np.float32)
    w_chord = np.asarray(inputs["w_chord"], dtype=np.float32)
    w_beat = np.asarray(inputs["w_beat"], dtype=np.float32)
    w_mel = np.asarray(inputs["w_mel"], dtype=np.float32)
    b_heads = np.concatenate([
        np.asarray(inputs["b_chord"], dtype=np.float32),
        np.asarray(inputs["b_beat"], dtype=np.float32),
        np.asarray(inputs["b_mel"], dtype=np.float32),
    ])
    genre = int(np.asarray(inputs["genre"]).reshape(-1)[0])
    tempo = int(np.asarray(inputs["tempo"]).reshape(-1)[0])
    key_sig = int(np.asarray(inputs["key_sig"]).reshape(-1)[0])

    # Fold conv into head weights: W'[e] = k0*W[e+1] + k1*W[e] + k2*W[e-1]
    W = np.concatenate([w_chord, w_beat, w_mel], axis=1)  # [50937, 168]
    k0, k1, k2 = (float(v) for v in conv_w[0, 0, 1, :])
    Wp = k1 * W
    Wp[:-1] += k0 * W[1:]
    Wp[1:] += k2 * W[:-1]

    # Bias: head biases + conv bias * colsum(W) + context-embedding term
    ids = [genre, 10 + tempo, 20 + key_sig, 34]
    ctx = emb[ids].sum(axis=0).astype(np.float64)  # [256]
    bias = (
        b_heads.astype(np.float64)
        + float(conv_b[0]) * W.sum(axis=0, dtype=np.float64)
        + ctx @ Wp[0:256].astype(np.float64)
    )  # [168]

    # Device operands: xT [51200, 2048] (zero padded), W' rows 256.. packed
    # [128, kt*168] with per-k-tile head-weight blocks
    K_PAD = N_CORES * K_PER
    XT = np.zeros((K_PAD, T), np.float16)
    XT[0:256] = melody.T
    XT[256:K_GEMM] = lyrics.T
    Wg = np.zeros((K_PAD, N_OUT), np.float16)
    Wg[0:K_GEMM] = Wp[256:]

    in_maps = []
    for c in range(N_CORES):
        wslab = Wg[c * K_PER:(c + 1) * K_PER]
        wc = np.zeros((128, KT * N_OUT), np.float16)
        wc[:, :49 * N_OUT] = (
            wslab[:49 * 128]
            .reshape(49, 128, N_OUT)
            .transpose(1, 0, 2)
            .reshape(128, 49 * N_OUT)
        )
        wc[0:64, 49 * N_OUT:] = wslab[49 * 128:]
        in_maps.append({
            "xt": XT[c * K_PER:(c + 1) * K_PER],
            "w": wc,
        })

    trace = bool(os.environ.get("HARMONY_TRACE"))
    res = run_bass_kernel_spmd(_get_nc(), in_maps, core_ids=list(range(N_CORES)), trace=trace)
    LAST_RESULT = res

    acc = np.zeros((N_OUT, T), np.float64)
    for r in res.results:
        acc += r["out"].astype(np.float64)
    out = (acc + bias[:, None]).T
    return np.ascontiguousarray(out.astype(np.float32))


# revision 34
# speedup vs baseline: 1.0515x; 1.0515x over previous
"""HarmonyGenerator Trainium2 kernel.

Math: the reference's 3x3 conv on [T,1,1,D] degenerates to a 3-tap conv along
the feature axis (only the kernel's middle row touches data).  Conv and the
three linear heads are both linear, so the conv folds into the head weights
(W' = 3-tap correlation of W along K) and the constant context-embedding rows
plus conv bias fold into the output bias.  The device work is one GEMM:

    out[2048, 168] = [melody | lyrics][2048, 50681] @ W'[50681, 168] + bias

Sharding: K (feature) axis split 8 ways, 6336 rows per core (49 full
128-row k-tiles + one 64-row tail tile; 8*6336 = 50688 >= 50681, 7 pad rows
on the last core only).  Each core reads 1/8 of x AND 1/8 of W and produces
a partial [168, 2048] (fp16); partials are summed on the host during the
gather/unshard step.

Schedule (v5): the PE's work rate (1296 ns/k-tile: four full-width 512-col
matmul streams for the first 128 out-cols + two concurrent 40-col pairs) is
~7% faster than the 16-queue DMA delivery rate (~1387 ns/k-tile at the
~409 B/ns aggregate wire speed), so the kernel should finish one chunk after
the last x byte.  What breaks that is delivery ORDER: each HWDGE ring is
FIFO with ~4 DIRECT2D issue credits, and the SDMA engines split bandwidth
~50/50 between the rings, so when two consecutively-needed chunks sit on
the same ring the PE waits ~5-12 us for the second one while the other ring
streams bytes it won't need for a while; the stall also re-trips the HAM
clock gate (>3.4 us PE-idle drops the array to 1.2 GHz for the next ~3.4 us
of work).  v5 therefore issues x as uniform 2-k-tile (1 MB) chunks strictly
alternating between the rings, and splits every W chunk into two half-DMAs
(one per ring), so both rings drain in lock-step with PE need order and
every chunk lands at the full aggregate rate.  A 24-matmul warm-up burst
(~7 us) delays the real stream until a few chunks of backlog are resident
and keeps the array active from the first cycle, so HAM never re-gates.
The PE consumes chunks in 2-chunk groups (the 40-col pairs of the previous
group run first and fill any arrival wait; the PE is in-order, so filler
placed after a stalling matmul could never slide forward).  The tail keeps
the last chunks at 1 k-tile and interleaves the kt-49 matmuls with PSUM
evictions so only ~2 us of work trails the last byte.

Fixed costs in the measured window: ~1.3 us of framework preamble before
the tile body can issue its first DMA, and ~13 us after the last real
matmul (eviction/store tail, exit barrier, then the NEFF's fixed
254-semaphore clear chain, whose slowest engine -- the Tensor NX at
~131 ns/clear -- defines the measured exec end).
"""

import os
import numpy as np

import concourse.bacc as bacc
import concourse.mybir as mybir
from concourse.tile import TileContext
from concourse.bass_utils import run_bass_kernel_spmd

# Problem shapes (hardcoded per contract)
T = 2048               # steps = length * 128
D_IN = 50937           # 256 ctx + 256 melody/vel + 50425 lyrics
K_GEMM = 50681         # melody(256) + lyrics(50425) features in the GEMM
N_OUT = 168            # 24 chord + 16 beat + 128 mel
N_CORES = 8
K_PER = 6336           # per-core K rows (49*128 + 64; 8*6336 = 50688 >= 50681)
KT = 50                # k-tiles per core (49 full 128-row + one 64-row tail)
TB = 512               # t-block (max fp32 moving dim / PSUM bank)
NTB = T // TB          # 4
KT_LAST = KT - 1

# All warm-up up front: front warm-up is free on delivery-paced (slow-HBM)
# draws -- the backlog builds while it runs -- whereas dummies interleaved
# into the stream tax those draws whenever the stalls land elsewhere.
WARMUP_MMS = 24
RAMP_FILL = {}

# x chunks (start_kt, n_kt): uniform 1 MB chunks, strictly alternating
# between the two HWDGE rings so delivery order tracks PE need order
# (smaller chunks starve the ~4-credit-deep ring issue pipeline and cost
# ~5% of delivery rate; larger ones make the arrival frontier lumpy);
# 1-k-tile tail so little work trails the last byte.
X_CHUNKS = [(2 * i, 2) for i in range(24)] + [(48, 1), (49, 1)]
# W chunks are split into two half-DMAs, one per ring, so they don't skew
# the rings' byte balance.  Each lands a couple of x chunks ahead of need.
W_CHUNKS = [(0, 4), (4, 6), (10, 8), (18, 8), (26, 8), (34, 8), (42, 8)]
# issue order: W chunk j is needed from x chunk [0, 2, 5, 9, 13, 17, 21].
ORDER = (
    [('w', 0), ('x', 0), ('x', 1), ('w', 1), ('x', 2), ('x', 3), ('w', 2),
     ('x', 4), ('x', 5), ('x', 6), ('x', 7), ('w', 3), ('x', 8), ('x', 9),
     ('x', 10), ('x', 11), ('w', 4), ('x', 12), ('x', 13), ('x', 14),
     ('x', 15), ('w', 5), ('x', 16), ('x', 17), ('x', 18), ('x', 19),
     ('w', 6), ('x', 20), ('x', 21), ('x', 22), ('x', 23), ('x', 24),
     ('x', 25)]
)
# PE consumption groups (indices into X_CHUNKS): fine in the ramp, paired in
# the body to amortize the ~166 ns column-group reconfig per cb<->mel swap
# (wider groups make the chunk-arrival frontier lumpier and stall the PE at
# group boundaries).
PE_GROUPS = ([[0], [1], [2], [3]] + [[i, i + 1] for i in range(4, 24, 2)]
             + [[24], [25]])

assert sum(n for _, n in X_CHUNKS) == KT
assert sum(n for _, n in W_CHUNKS) == KT
assert [i for g in PE_GROUPS for i in g] == list(range(len(X_CHUNKS)))

_NC = None
LAST_RESULT = None     # BassKernelResults of the most recent run (for test.py)


def _build_nc():
    f32 = mybir.dt.float32
    f16 = mybir.dt.float16
    nc = bacc.Bacc()
    # xt is plain [K, T]: each chunk is then one contiguous 1 MB DRAM extent
    # (a k-tile-major partition-planar pack was tried and is ~2% slower --
    # it turns a chunk into 128 strided 8 KB extents, which HBM likes less).
    xt = nc.dram_tensor("xt", [K_PER, T], f16, kind="ExternalInput")
    w = nc.dram_tensor("w", [128, KT * N_OUT], f16, kind="ExternalInput")
    out = nc.dram_tensor("out", [N_OUT, T], f16, kind="ExternalOutput")

    with TileContext(nc) as tc:
        with (
            tc.tile_pool(name="wp", bufs=1) as wp,
            tc.tile_pool(name="xp", bufs=1) as xp,
            tc.tile_pool(name="op", bufs=2) as op,
            tc.tile_pool(name="ps", bufs=1, space="PSUM") as ps,
        ):
            # HAM warm-up: the PE clock-gate holds matmuls at 1.2 GHz until
            # ~3.4us of sustained activity, and the first chunks need a few
            # us to land.  Burn that window on dummy matmuls so the real
            # stream starts warm with delivered backlog and never pauses
            # long enough to re-gate.
            dm = wp.tile([128, TB], f16, tag="warm", name="warmup")
            nc.vector.memset(dm[:], 0.0)
            ps_warm = ps.tile([128, TB], f32, tag="warm_ps", name="ps_warm")
            for _ in range(WARMUP_MMS):
                nc.tensor.matmul(ps_warm[:], dm[:, 0:128], dm[:], start=True, stop=True)

            # Build W tiles and x tiles; issue DMAs on the two HWDGE rings.
            # x chunk i goes to ring i%2; W chunks go half to each ring.
            w_of = {}
            x_of = {}
            rings = [nc.sync, nc.scalar]
            for kind, idx in ORDER:
                if kind == 'w':
                    s, n = W_CHUNKS[idx]
                    tile = wp.tile([128, n * N_OUT], f16, tag=f"w{idx}", name=f"w{idx}")
                    h = n // 2
                    rings[0].dma_start(
                        tile[:, 0:h * N_OUT],
                        w[:, s * N_OUT:(s + h) * N_OUT])
                    rings[1].dma_start(
                        tile[:, h * N_OUT:n * N_OUT],
                        w[:, (s + h) * N_OUT:(s + n) * N_OUT])
                    for j in range(n):
                        w_of[s + j] = (tile, j * N_OUT)
                else:
                    s, n = X_CHUNKS[idx]
                    r = idx % 2
                    if s == KT_LAST:
                        # 64-row tail k-tile (K rows 6272..6335)
                        tile = xp.tile([64, T], f16, tag="xtail", name="xtail",
                                       bufs=1)
                        rings[r].dma_start(tile[:], xt[s * 128:s * 128 + 64, :])
                    elif n == 1:
                        tile = xp.tile([128, T], f16, tag="x1", name=f"x1_{idx}",
                                       bufs=2)
                        rings[r].dma_start(tile[:], xt[s * 128:(s + 1) * 128, :])
                    else:
                        tile = xp.tile([128, n * T], f16, tag=f"x{n}",
                                       name=f"x{n}_{idx}", bufs=14)
                        rings[r].dma_start(
                            tile[:].rearrange("p (a t) -> p a t", a=n),
                            xt[s * 128:(s + n) * 128, :].rearrange(
                                "(a p) t -> p a t", p=128),
                        )
                    for j in range(n):
                        x_of[s + j] = (tile, j * T)

            # Persistent accumulators: 4 mel banks + 2 shared cb banks.  Each
            # cb bank holds two t-blocks' [40, TB] outputs col-tiled into
            # partitions 0:40 and 64:104 (concurrent matmuls via tile_position).
            psm = [ps.tile([128, TB], f32, tag=f"m{t}", name=f"psm{t}") for t in range(NTB)]
            psc = [ps.tile([128, TB], f32, tag=f"c{p}", name=f"psc{p}") for p in range(NTB // 2)]

            def krows(kt):
                return 64 if kt == KT_LAST else 128

            def rhs_of(kt, t):
                tile, off = x_of[kt]
                return tile[0:krows(kt), off + t * TB: off + (t + 1) * TB]

            def mel(kt, t):
                wt, j = w_of[kt]
                nc.tensor.matmul(psm[t][:], wt[0:krows(kt), j: j + 128],
                                 rhs_of(kt, t),
                                 start=(kt == 0), stop=(kt == KT_LAST))

            def cb_pair(kt, p):
                wt, j = w_of[kt]
                lhs_c = wt[0:krows(kt), j + 128: j + N_OUT]
                nc.tensor.matmul(psc[p][0:40, :], lhs_c, rhs_of(kt, 2 * p),
                                 start=(kt == 0), stop=(kt == KT_LAST),
                                 tile_position=(0, 0))
                nc.tensor.matmul(psc[p][64:104, :], lhs_c, rhs_of(kt, 2 * p + 1),
                                 start=(kt == 0), stop=(kt == KT_LAST),
                                 tile_position=(0, 64))

            # psm evictions stage t-block pairs into one [128, 2*TB] tile so
            # the out DMA issues once per pair: each DMA_DIRECT2D costs
            # ~0.7 us of descriptor generation on its ring engine, and the
            # tail is issue-bound, not transfer-bound.
            om = [op.tile([128, 2 * TB], f16, tag=f"om{h}", name=f"om{h}", bufs=1)
                  for h in range(2)]

            def evict_mel(t):
                h, half = t // 2, t % 2
                o = om[h][:, half * TB:(half + 1) * TB]
                if half == 0:
                    nc.vector.tensor_copy(o, psm[t][:])
                else:
                    nc.scalar.copy(o, psm[t][:])
                    ring = nc.sync if h == 0 else nc.scalar
                    ring.dma_start(out[0:128, h * 2 * TB:(h + 1) * 2 * TB], om[h][:])

            def evict_cb(p):
                # cb out DMAs go on the SWDGE ring: input traffic is done by
                # now, and each DMA_DIRECT2D issue costs ~0.7 us on its
                # engine -- spreading the eviction issues over three engines
                # (sync/scalar for mel, gpsimd for cb) shortens the tail.
                o = op.tile([104, TB], f16, tag="oc", name=f"oc{p}")
                if p == 0:
                    nc.vector.tensor_copy(o[0:104, :], psc[p][0:104, :])
                else:
                    nc.scalar.copy(o[0:104, :], psc[p][0:104, :])
                nc.gpsimd.dma_start(out[128:N_OUT, 2 * p * TB:(2 * p + 1) * TB], o[0:40, :])
                ring = nc.sync if p == 0 else nc.scalar
                ring.dma_start(out[128:N_OUT, (2 * p + 1) * TB:(2 * p + 2) * TB], o[64:104, :])

            # The cb pairs of group g run right before the mels of group
            # g+1: pairs touch only already-resident data, so when the mels
            # would stall on a fresh chunk arrival the PE fills the wait
            # with pair work instead of idling (the PE is in-order, so
            # pairs placed after stalling mels could never slide forward).
            def kts_of(group):
                return [kt for ci in group
                        for kt in range(X_CHUNKS[ci][0],
                                        X_CHUNKS[ci][0] + X_CHUNKS[ci][1])]

            for gi, group in enumerate(PE_GROUPS):
                if gi > 0:
                    for kt in kts_of(PE_GROUPS[gi - 1]):
                        cb_pair(kt, 0)
                        cb_pair(kt, 1)
                for _ in range(RAMP_FILL.get(gi, 0)):
                    nc.tensor.matmul(ps_warm[:], dm[:, 0:128], dm[:],
                                     start=True, stop=True)
                if gi < len(PE_GROUPS) - 1:
                    for kt in kts_of(group):
                        for t in range(NTB):
                            mel(kt, t)
                else:
                    # kt49: cb pairs first so their evictions overlap the
                    # mel evictions; then mel bank-by-bank with evictions
                    # interleaved so little work trails the last matmul.
                    cb_pair(KT_LAST, 0)
                    cb_pair(KT_LAST, 1)
                    mel(KT_LAST, 0)
                    evict_mel(0)
                    mel(KT_LAST, 1)
                    evict_mel(1)
                    evict_cb(0)
                    mel(KT_LAST, 2)
                    evict_mel(2)
                    mel(KT_LAST, 3)
                    evict_mel(3)
                    evict_cb(1)
    return nc


def _get_nc():
    global _NC
    if _NC is None:
        _NC = _build_nc()
        if not _NC.is_finalized():
            _NC.finalize()
    return _NC


def kernel(**inputs):
    global LAST_RESULT
    melody = np.ascontiguousarray(np.asarray(inputs["melody_tensor"], dtype=np.float32))
    lyrics = np.ascontiguousarray(np.asarray(inputs["lyrics_tensor"], dtype=np.float32))
    emb = np.asarray(inputs["emb"], dtype=np.float32)
    conv_w = np.asarray(inputs["conv_w"], dtype=np.float32)
    conv_b = np.asarray(inputs["conv_b"], dtype=np.float32)
    w_chord = np.asarray(inputs["w_chord"], dtype=np.float32)
    w_beat = np.asarray(inputs["w_beat"], dtype=np.float32)
    w_mel = np.asarray(inputs["w_mel"], dtype=np.float32)
    b_heads = np.concatenate([
        np.asarray(inputs["b_chord"], dtype=np.float32),
        np.asarray(inputs["b_beat"], dtype=np.float32),
        np.asarray(inputs["b_mel"], dtype=np.float32),
    ])
    genre = int(np.asarray(inputs["genre"]).reshape(-1)[0])
    tempo = int(np.asarray(inputs["tempo"]).reshape(-1)[0])
    key_sig = int(np.asarray(inputs["key_sig"]).reshape(-1)[0])

    # Fold conv into head weights: W'[e] = k0*W[e+1] + k1*W[e] + k2*W[e-1]
    W = np.concatenate([w_chord, w_beat, w_mel], axis=1)  # [50937, 168]
    k0, k1, k2 = (float(v) for v in conv_w[0, 0, 1, :])
    Wp = k1 * W
    Wp[:-1] += k0 * W[1:]
    Wp[1:] += k2 * W[:-1]

    # Bias: head biases + conv bias * colsum(W) + context-embedding term
    ids = [genre, 10 + tempo, 20 + key_sig, 34]
    ctx = emb[ids].sum(axis=0).astype(np.float64)  # [256]
    bias = (
        b_heads.astype(np.float64)
        + float(conv_b[0]) * W.sum(axis=0, dtype=np.float64)
        + ctx @ Wp[0:256].astype(np.float64)
    )  # [168]

    # Device operands: xT [51200, 2048] (zero padded), W' rows 256.. packed
    # [128, kt*168] with per-k-tile head-weight blocks
    K_PAD = N_CORES * K_PER
    XT = np.zeros((K_PAD, T), np.float16)
    XT[0:256] = melody.T
    XT[256:K_GEMM] = lyrics.T
    Wg = np.zeros((K_PAD, N_OUT), np.float16)
    Wg[0:K_GEMM] = Wp[256:]

    in_maps = []
    for c in range(N_CORES):
        wslab = Wg[c * K_PER:(c + 1) * K_PER]
        wc = np.zeros((128, KT * N_OUT), np.float16)
        wc[:, :49 * N_OUT] = (
            wslab[:49 * 128]
            .reshape(49, 128, N_OUT)
            .transpose(1, 0, 2)
            .reshape(128, 49 * N_OUT)
        )
        wc[0:64, 49 * N_OUT:] = wslab[49 * 128:]
        in_maps.append({
            "xt": XT[c * K_PER:(c + 1) * K_PER],
            "w": wc,
        })

    trace = bool(os.environ.get("HARMONY_TRACE"))
    res = run_bass_kernel_spmd(_get_nc(), in_maps, core_ids=list(range(N_CORES)), trace=trace)
    LAST_RESULT = res

    acc = np.zeros((N_OUT, T), np.float64)
    for r in res.results:
        acc += r["out"].astype(np.float64)
    out = (acc + bias[:, None]).T
    return np.ascontiguousarray(out.astype(np.float32))


# revision 37
# speedup vs baseline: 1.0989x; 1.0451x over previous
"""HarmonyGenerator Trainium2 kernel.

Math: the reference's 3x3 conv on [T,1,1,D] degenerates to a 3-tap conv along
the feature axis (only the kernel's middle row touches data).  Conv and the
three linear heads are both linear, so the conv folds into the head weights
(W' = 3-tap correlation of W along K) and the constant context-embedding rows
plus conv bias fold into the output bias.  The device work is one GEMM:

    out[2048, 168] = [melody | lyrics][2048, 50681] @ W'[50681, 168] + bias

Sharding: K (feature) axis split 8 ways, 6336 rows per core (49 full
128-row k-tiles + one 64-row tail tile; 8*6336 = 50688 >= 50681, 7 pad rows
on the last core only).  Each core reads 1/8 of x AND 1/8 of W and produces
a partial [168, 2048] (fp16); partials are summed on the host during the
gather/unshard step.

Schedule (v5): the PE's work rate (1296 ns/k-tile: four full-width 512-col
matmul streams for the first 128 out-cols + two concurrent 40-col pairs) is
~7% faster than the 16-queue DMA delivery rate (~1387 ns/k-tile at the
~409 B/ns aggregate wire speed), so the kernel should finish one chunk after
the last x byte.  What breaks that is delivery ORDER: each HWDGE ring is
FIFO with ~4 DIRECT2D issue credits, and the SDMA engines split bandwidth
~50/50 between the rings, so when two consecutively-needed chunks sit on
the same ring the PE waits ~5-12 us for the second one while the other ring
streams bytes it won't need for a while; the stall also re-trips the HAM
clock gate (>3.4 us PE-idle drops the array to 1.2 GHz for the next ~3.4 us
of work).  v5 therefore issues x as uniform 2-k-tile (1 MB) chunks strictly
alternating between the rings, and splits every W chunk into two half-DMAs
(one per ring), so both rings drain in lock-step with PE need order and
every chunk lands at the full aggregate rate.  A 24-matmul warm-up burst
(~7 us) delays the real stream until a few chunks of backlog are resident
and keeps the array active from the first cycle, so HAM never re-gates.
The PE consumes chunks in 2-chunk groups (the 40-col pairs of the previous
group run first and fill any arrival wait; the PE is in-order, so filler
placed after a stalling matmul could never slide forward).  The tail keeps
the last chunks at 1 k-tile and interleaves the kt-49 matmuls with PSUM
evictions so only ~2 us of work trails the last byte.

Fixed costs in the measured window: ~1.3 us of framework preamble before
the tile body can issue its first DMA, and ~13 us after the last real
matmul (eviction/store tail, exit barrier, then the NEFF's fixed
254-semaphore clear chain, whose slowest engine -- the Tensor NX at
~131 ns/clear -- defines the measured exec end).
"""

import os
import numpy as np

import concourse.bacc as bacc
import concourse.mybir as mybir
from concourse.tile import TileContext
from concourse.bass_utils import run_bass_kernel_spmd

# Problem shapes (hardcoded per contract)
T = 2048               # steps = length * 128
D_IN = 50937           # 256 ctx + 256 melody/vel + 50425 lyrics
K_GEMM = 50681         # melody(256) + lyrics(50425) features in the GEMM
N_OUT = 168            # 24 chord + 16 beat + 128 mel
N_CORES = 8
K_PER = 6336           # per-core K rows (49*128 + 64; 8*6336 = 50688 >= 50681)
KT = 50                # k-tiles per core (49 full 128-row + one 64-row tail)
TB = 512               # t-block (max fp32 moving dim / PSUM bank)
NTB = T // TB          # 4
KT_LAST = KT - 1

# All warm-up up front: front warm-up is free on delivery-paced (slow-HBM)
# draws -- the backlog builds while it runs -- whereas dummies interleaved
# into the stream tax those draws whenever the stalls land elsewhere.
WARMUP_MMS = 24
RAMP_FILL = {}

# x chunks (start_kt, n_kt): uniform 1 MB chunks, strictly alternating
# between the two HWDGE rings so delivery order tracks PE need order
# (smaller chunks starve the ~4-credit-deep ring issue pipeline and cost
# ~5% of delivery rate; larger ones make the arrival frontier lumpy);
# 1-k-tile tail so little work trails the last byte.
X_CHUNKS = [(2 * i, 2) for i in range(24)] + [(48, 1), (49, 1)]
# W chunks are split into two half-DMAs, one per ring, so they don't skew
# the rings' byte balance.  The first two are tiny (2 k-tiles) so the
# delivery-critical ramp window carries as little W as possible; each chunk
# lands a couple of x chunks ahead of need.
W_CHUNKS = [(0, 2), (2, 2), (4, 6), (10, 8), (18, 8), (26, 8), (34, 8),
            (42, 8)]
# issue order: W chunk j is needed from x chunk [0, 1, 2, 5, 9, 13, 17, 21].
ORDER = (
    [('w', 0), ('x', 0), ('x', 1), ('w', 1), ('x', 2), ('w', 2), ('x', 3),
     ('x', 4), ('w', 3), ('x', 5), ('x', 6), ('x', 7), ('x', 8), ('w', 4),
     ('x', 9), ('x', 10), ('x', 11), ('x', 12), ('w', 5), ('x', 13),
     ('x', 14), ('x', 15), ('x', 16), ('w', 6), ('x', 17), ('x', 18),
     ('x', 19), ('x', 20), ('w', 7), ('x', 21), ('x', 22), ('x', 23),
     ('x', 24), ('x', 25)]
)
# PE consumption groups (indices into X_CHUNKS): fine in the ramp, paired in
# the body to amortize the ~166 ns column-group reconfig per cb<->mel swap
# (wider groups make the chunk-arrival frontier lumpier and stall the PE at
# group boundaries).
PE_GROUPS = ([[0], [1], [2], [3]] + [[i, i + 1] for i in range(4, 24, 2)]
             + [[24], [25]])

assert sum(n for _, n in X_CHUNKS) == KT
assert sum(n for _, n in W_CHUNKS) == KT
assert [i for g in PE_GROUPS for i in g] == list(range(len(X_CHUNKS)))

_NC = None
LAST_RESULT = None     # BassKernelResults of the most recent run (for test.py)


def _build_nc():
    f32 = mybir.dt.float32
    f16 = mybir.dt.float16
    nc = bacc.Bacc()
    # xt is plain [K, T]: each chunk is then one contiguous 1 MB DRAM extent
    # (a k-tile-major partition-planar pack was tried and is ~2% slower --
    # it turns a chunk into 128 strided 8 KB extents, which HBM likes less).
    xt = nc.dram_tensor("xt", [K_PER, T], f16, kind="ExternalInput")
    w = nc.dram_tensor("w", [128, KT * N_OUT], f16, kind="ExternalInput")
    out = nc.dram_tensor("out", [N_OUT, T], f16, kind="ExternalOutput")

    with TileContext(nc) as tc:
        with (
            tc.tile_pool(name="wp", bufs=1) as wp,
            tc.tile_pool(name="xp", bufs=1) as xp,
            tc.tile_pool(name="op", bufs=2) as op,
            tc.tile_pool(name="ps", bufs=1, space="PSUM") as ps,
        ):
            # HAM warm-up: the PE clock-gate holds matmuls at 1.2 GHz until
            # ~3.4us of sustained activity, and the first chunks need a few
            # us to land.  Burn that window on dummy matmuls so the real
            # stream starts warm with delivered backlog and never pauses
            # long enough to re-gate.
            dm = wp.tile([128, TB], f16, tag="warm", name="warmup")
            nc.vector.memset(dm[:], 0.0)
            ps_warm = ps.tile([128, TB], f32, tag="warm_ps", name="ps_warm")
            for _ in range(WARMUP_MMS):
                nc.tensor.matmul(ps_warm[:], dm[:, 0:128], dm[:], start=True, stop=True)

            # Build W tiles and x tiles; issue DMAs on the two HWDGE rings.
            # x chunk i goes to ring i%2; W chunks go half to each ring.
            w_of = {}
            x_of = {}
            rings = [nc.sync, nc.scalar]
            for kind, idx in ORDER:
                if kind == 'w':
                    s, n = W_CHUNKS[idx]
                    tile = wp.tile([128, n * N_OUT], f16, tag=f"w{idx}", name=f"w{idx}")
                    h = n // 2
                    rings[0].dma_start(
                        tile[:, 0:h * N_OUT],
                        w[:, s * N_OUT:(s + h) * N_OUT])
                    rings[1].dma_start(
                        tile[:, h * N_OUT:n * N_OUT],
                        w[:, (s + h) * N_OUT:(s + n) * N_OUT])
                    for j in range(n):
                        w_of[s + j] = (tile, j * N_OUT)
                else:
                    s, n = X_CHUNKS[idx]
                    r = idx % 2
                    if s == KT_LAST:
                        # 64-row tail k-tile (K rows 6272..6335)
                        tile = xp.tile([64, T], f16, tag="xtail", name="xtail",
                                       bufs=1)
                        rings[r].dma_start(tile[:], xt[s * 128:s * 128 + 64, :])
                    elif n == 1:
                        tile = xp.tile([128, T], f16, tag="x1", name=f"x1_{idx}",
                                       bufs=2)
                        rings[r].dma_start(tile[:], xt[s * 128:(s + 1) * 128, :])
                    else:
                        tile = xp.tile([128, n * T], f16, tag=f"x{n}",
                                       name=f"x{n}_{idx}", bufs=14)
                        rings[r].dma_start(
                            tile[:].rearrange("p (a t) -> p a t", a=n),
                            xt[s * 128:(s + n) * 128, :].rearrange(
                                "(a p) t -> p a t", p=128),
                        )
                    for j in range(n):
                        x_of[s + j] = (tile, j * T)

            # Persistent accumulators: 4 mel banks + 2 shared cb banks.  Each
            # cb bank holds two t-blocks' [40, TB] outputs col-tiled into
            # partitions 0:40 and 64:104 (concurrent matmuls via tile_position).
            psm = [ps.tile([128, TB], f32, tag=f"m{t}", name=f"psm{t}") for t in range(NTB)]
            psc = [ps.tile([128, TB], f32, tag=f"c{p}", name=f"psc{p}") for p in range(NTB // 2)]

            def krows(kt):
                return 64 if kt == KT_LAST else 128

            def rhs_of(kt, t):
                tile, off = x_of[kt]
                return tile[0:krows(kt), off + t * TB: off + (t + 1) * TB]

            def mel(kt, t):
                wt, j = w_of[kt]
                nc.tensor.matmul(psm[t][:], wt[0:krows(kt), j: j + 128],
                                 rhs_of(kt, t),
                                 start=(kt == 0), stop=(kt == KT_LAST))

            def cb_pair(kt, p):
                wt, j = w_of[kt]
                lhs_c = wt[0:krows(kt), j + 128: j + N_OUT]
                nc.tensor.matmul(psc[p][0:40, :], lhs_c, rhs_of(kt, 2 * p),
                                 start=(kt == 0), stop=(kt == KT_LAST),
                                 tile_position=(0, 0))
                nc.tensor.matmul(psc[p][64:104, :], lhs_c, rhs_of(kt, 2 * p + 1),
                                 start=(kt == 0), stop=(kt == KT_LAST),
                                 tile_position=(0, 64))

            # psm evictions: one staging tile + one out DMA per PSUM bank.
            om = [op.tile([128, TB], f16, tag=f"om{t}", name=f"om{t}", bufs=1)
                  for t in range(NTB)]

            def evict_mel(t):
                # one store per PSUM bank, issued the moment its copy lands
                # and alternating rings: the last store on the exit-barrier
                # critical path is 128 KB instead of 256 KB and starts a
                # copy earlier.
                if t % 2 == 0:
                    nc.vector.tensor_copy(om[t][:], psm[t][:])
                else:
                    nc.scalar.copy(om[t][:], psm[t][:])
                rings[t % 2].dma_start(out[0:128, t * TB:(t + 1) * TB], om[t][:])

            def evict_cb(p):
                # cb out DMAs go on the SWDGE ring: input traffic is done by
                # now, and each DMA_DIRECT2D issue costs ~0.7 us on its
                # engine -- spreading the eviction issues over three engines
                # (sync/scalar for mel, gpsimd for cb) shortens the tail.
                o = op.tile([104, TB], f16, tag="oc", name=f"oc{p}")
                if p == 0:
                    nc.vector.tensor_copy(o[0:104, :], psc[p][0:104, :])
                else:
                    nc.scalar.copy(o[0:104, :], psc[p][0:104, :])
                nc.gpsimd.dma_start(out[128:N_OUT, 2 * p * TB:(2 * p + 1) * TB], o[0:40, :])
                ring = nc.sync if p == 0 else nc.scalar
                ring.dma_start(out[128:N_OUT, (2 * p + 1) * TB:(2 * p + 2) * TB], o[64:104, :])

            # The cb pairs of group g run right before the mels of group
            # g+1: pairs touch only already-resident data, so when the mels
            # would stall on a fresh chunk arrival the PE fills the wait
            # with pair work instead of idling (the PE is in-order, so
            # pairs placed after stalling mels could never slide forward).
            def kts_of(group):
                return [kt for ci in group
                        for kt in range(X_CHUNKS[ci][0],
                                        X_CHUNKS[ci][0] + X_CHUNKS[ci][1])]

            for gi, group in enumerate(PE_GROUPS):
                if gi > 0:
                    for kt in kts_of(PE_GROUPS[gi - 1]):
                        cb_pair(kt, 0)
                        cb_pair(kt, 1)
                for _ in range(RAMP_FILL.get(gi, 0)):
                    nc.tensor.matmul(ps_warm[:], dm[:, 0:128], dm[:],
                                     start=True, stop=True)
                if gi < len(PE_GROUPS) - 1:
                    for kt in kts_of(group):
                        for t in range(NTB):
                            mel(kt, t)
                else:
                    # kt49: cb pairs first so their evictions overlap the
                    # mel evictions; then mel bank-by-bank with evictions
                    # interleaved so little work trails the last matmul.
                    cb_pair(KT_LAST, 0)
                    cb_pair(KT_LAST, 1)
                    mel(KT_LAST, 0)
                    evict_mel(0)
                    mel(KT_LAST, 1)
                    evict_mel(1)
                    evict_cb(0)
                    mel(KT_LAST, 2)
                    evict_mel(2)
                    mel(KT_LAST, 3)
                    evict_mel(3)
                    evict_cb(1)
    return nc


def _get_nc():
    global _NC
    if _NC is None:
        _NC = _build_nc()
        if not _NC.is_finalized():
            _NC.finalize()
    return _NC


def kernel(**inputs):
    global LAST_RESULT
    melody = np.ascontiguousarray(np.asarray(inputs["melody_tensor"], dtype=np.float32))
    lyrics = np.ascontiguousarray(np.asarray(inputs["lyrics_tensor"], dtype=np.float32))
    emb = np.asarray(inputs["emb"], dtype=np.float32)
    conv_w = np.asarray(inputs["conv_w"], dtype=np.float32)
    conv_b = np.asarray(inputs["conv_b"], dtype=np.float32)
    w_chord = np.asarray(inputs["w_chord"], dtype=np.float32)
    w_beat = np.asarray(inputs["w_beat"], dtype=np.float32)
    w_mel = np.asarray(inputs["w_mel"], dtype=np.float32)
    b_heads = np.concatenate([
        np.asarray(inputs["b_chord"], dtype=np.float32),
        np.asarray(inputs["b_beat"], dtype=np.float32),
        np.asarray(inputs["b_mel"], dtype=np.float32),
    ])
    genre = int(np.asarray(inputs["genre"]).reshape(-1)[0])
    tempo = int(np.asarray(inputs["tempo"]).reshape(-1)[0])
    key_sig = int(np.asarray(inputs["key_sig"]).reshape(-1)[0])

    # Fold conv into head weights: W'[e] = k0*W[e+1] + k1*W[e] + k2*W[e-1]
    W = np.concatenate([w_chord, w_beat, w_mel], axis=1)  # [50937, 168]
    k0, k1, k2 = (float(v) for v in conv_w[0, 0, 1, :])
    Wp = k1 * W
    Wp[:-1] += k0 * W[1:]
    Wp[1:] += k2 * W[:-1]

    # Bias: head biases + conv bias * colsum(W) + context-embedding term
    ids = [genre, 10 + tempo, 20 + key_sig, 34]
    ctx = emb[ids].sum(axis=0).astype(np.float64)  # [256]
    bias = (
        b_heads.astype(np.float64)
        + float(conv_b[0]) * W.sum(axis=0, dtype=np.float64)
        + ctx @ Wp[0:256].astype(np.float64)
    )  # [168]

    # Device operands: xT [51200, 2048] (zero padded), W' rows 256.. packed
    # [128, kt*168] with per-k-tile head-weight blocks
    K_PAD = N_CORES * K_PER
    XT = np.zeros((K_PAD, T), np.float16)
    XT[0:256] = melody.T
    XT[256:K_GEMM] = lyrics.T
    Wg = np.zeros((K_PAD, N_OUT), np.float16)
    Wg[0:K_GEMM] = Wp[256:]

    in_maps = []
    for c in range(N_CORES):
        wslab = Wg[c * K_PER:(c + 1) * K_PER]
        wc = np.zeros((128, KT * N_OUT), np.float16)
        wc[:, :49 * N_OUT] = (
            wslab[:49 * 128]
            .reshape(49, 128, N_OUT)
            .transpose(1, 0, 2)
            .reshape(128, 49 * N_OUT)
        )
        wc[0:64, 49 * N_OUT:] = wslab[49 * 128:]
        in_maps.append({
            "xt": XT[c * K_PER:(c + 1) * K_PER],
            "w": wc,
        })

    trace = bool(os.environ.get("HARMONY_TRACE"))
    res = run_bass_kernel_spmd(_get_nc(), in_maps, core_ids=list(range(N_CORES)), trace=trace)
    LAST_RESULT = res

    acc = np.zeros((N_OUT, T), np.float64)
    for r in res.results:
        acc += r["out"].astype(np.float64)
    out = (acc + bias[:, None]).T
    return np.ascontiguousarray(out.astype(np.float32))
